# revision 2
# baseline (speedup 1.0000x reference)
"""GCNConv Bass kernel for Trainium2, 8-core SPMD.

Math (reference): out = D^-1/2 (A + I) D^-1/2 (x @ W) + b.
Aggregation commutes with the linear layer; with xs = dinv * x pre-scaled:
    out[d] = dinv[d] * ( sum_{e: dst(e)=d} xs[src(e)] + xs[d] ) @ W + b

Sharding: 256-dst windows are bin-packed across the 8 cores by edge count
(so the SPMD-shared group counts pad minimally).  Per core, edges are
bucketed by (src-band, window-slot), sorted by src, padded to 128-edge
groups with group counts shared across cores (padding edges have
dst_rel=-1 so they contribute nothing).  Source bands of 25000 rows exist
because dma_gather indices are int16.

xs is stored bf16 in a [N, 128] row (left half = features, right half
zero) so each gather descriptor is 256B (hardware minimum) yet messages
arrive in bf16 directly - the whole scatter pipeline runs at bf16 PE/DVE
rates.  Self loops are folded into the aggregation buffer's DMA-loaded
initial value (xsloc^T), eliminating the identity matmuls.

Device pipeline per 128-edge group:
  - dma_gather fetches 128 xs-rows -> msg[:, q, :] (SWDGE descriptor
    emission on GpSimd is the wall: ~2.7ns/edge, so everything else is
    sized to hide underneath it)
  - batched DVE is_equal builds bf16 one-hots for 16 groups at a time:
      ohT[e, b, d] = (iota[d] == dst_rel[e, b])
  - PE matmul accumulates agg_ps[64f, 256d] += msg^T @ ohT per cell
    (band x window-slot) in PSUM; DVE adds cells into the SBUF agg.
Final per 128-half-window: agg^T @ W matmul (fp32), dinv scale, bias,
output DMA.
"""

import numpy as np
import ml_dtypes

BF16 = ml_dtypes.bfloat16

N_NODES = 100000
N_FEAT = 64
N_CORES = 8
AW = 256  # aggregation window width (one-hot free dim)
WIN = 128  # final-matmul window (PE partition dim)
P = 128
BAND_ROWS = 25000  # int16 gather index limit (256B rows)
KG = 64  # max groups (of 128 edges) per dma_gather call
OH_B = 16  # groups per batched one-hot DVE instruction
N_QUEUES = 2


def _assign_windows(cnt_w, n_cores):
    """Bin-pack windows across cores by edge count: slot j holds the
    windows ranked [8j, 8j+8), one per core, so the max-over-cores group
    count at each slot stays near the mean.  Returns win_of[c][j] with -1
    for the dummy pad window."""
    nw = len(cnt_w)
    nslots = -(-nw // n_cores)
    order = np.argsort(-cnt_w, kind="stable")
    win_of = np.full((n_cores, nslots), -1, np.int64)
    for j in range(nslots):
        chunk = order[j * n_cores:(j + 1) * n_cores]
        for i, w in enumerate(chunk):
            win_of[(i + j) % n_cores, j] = w
    return win_of


def _prepare(x, edge_index, W, b, n_cores, band_rows, pad_neg1=True):
    N, C = x.shape
    n_bands = -(-N // band_rows)
    nw = -(-N // AW)  # global 256-dst windows
    nslots = -(-nw // n_cores)
    nwin = nslots * 2  # 128-wide final windows per core
    npc_out = nslots * AW

    row = np.asarray(edge_index[0], dtype=np.int64)
    col = np.asarray(edge_index[1], dtype=np.int64)

    deg = np.bincount(col, minlength=N) + 1  # +1 self loop
    dinv = (1.0 / np.sqrt(deg)).astype(np.float32)
    xs = np.asarray(x, dtype=np.float32) * dinv[:, None]

    w_glob = col // AW
    cnt_w = np.bincount(w_glob, minlength=nw)
    win_of = _assign_windows(cnt_w, n_cores)
    # inverse: window -> (core, slot)
    core_of_w = np.zeros(nw, np.int64)
    slot_of_w = np.zeros(nw, np.int64)
    for c in range(n_cores):
        for j in range(nslots):
            w = win_of[c, j]
            if w >= 0:
                core_of_w[w] = c
                slot_of_w[w] = j

    core = core_of_w[w_glob]
    slot = slot_of_w[w_glob]
    dst_rel = (col - w_glob * AW).astype(np.float32)
    band = row // band_rows

    order = np.lexsort((row, slot, band, core))
    row_s = row[order]
    band_s = band[order]
    rel_row_s = (row_s - band_s * band_rows).astype(np.int16)
    dr_s = dst_rel[order]

    key = (core[order] * n_bands + band_s) * nslots + slot[order]
    cnt = np.bincount(key, minlength=n_cores * n_bands * nslots).reshape(
        n_cores, n_bands, nslots)
    G_bw = (-(-cnt // P)).max(axis=0).astype(np.int64)  # [n_bands, nslots]
    gtot = int(G_bw.sum())

    gstart = np.zeros((n_bands, nslots), np.int64)
    gstart.reshape(-1)[1:] = np.cumsum(G_bw.reshape(-1))[:-1]

    # static per-band group metadata: (slot, k, Gslot) per group
    group_meta = []
    for bb in range(n_bands):
        gm = []
        for j in range(nslots):
            for k in range(int(G_bw[bb, j])):
                gm.append((j, k, int(G_bw[bb, j])))
        group_meta.append(gm)

    calls = []
    for bb in range(n_bands):
        g0 = int(gstart[bb, 0])
        gend = g0 + int(G_bw[bb].sum())
        g = g0
        while g < gend:
            ng = min(KG, gend - g)
            calls.append((bb, g, ng))
            g += ng

    estart = np.zeros(n_cores * n_bands * nslots + 1, np.int64)
    estart[1:] = np.cumsum(cnt.reshape(-1))

    # packed bf16 feature rows: [N, 2C]; right half zero so each gather
    # descriptor moves exactly 256B
    xs_pack = np.zeros((N, 2 * C), dtype=BF16)
    xs_pack[:, :C] = xs.astype(BF16)
    W32 = np.ascontiguousarray(np.asarray(W, dtype=np.float32))
    b32 = np.broadcast_to(np.asarray(b, dtype=np.float32), (P, C)).copy()

    pad_idx = -1 if pad_neg1 else 0  # -1: HW DGE skips the descriptor
    in_maps = []
    for c in range(n_cores):
        ridx = np.full((gtot, P), pad_idx, np.int16)
        drel = np.full((gtot, P), -1.0, np.float32)  # -1 => padding edge
        for bb in range(n_bands):
            for j in range(nslots):
                g0, gw = gstart[bb, j], G_bw[bb, j]
                if gw == 0:
                    continue
                k = (c * n_bands + bb) * nslots + j
                e0, e1 = estart[k], estart[k + 1]
                n_e = e1 - e0
                ridx[g0:g0 + gw].reshape(-1)[:n_e] = rel_row_s[e0:e1]
                drel[g0:g0 + gw].reshape(-1)[:n_e] = dr_s[e0:e1]
        if pad_neg1:
            # keep the final slot of every gather call valid (dst_rel stays
            # -1 so it contributes nothing) so no call ends all-skipped
            for _, cg0, cng in calls:
                if ridx[cg0 + cng - 1, P - 1] < 0:
                    ridx[cg0 + cng - 1, P - 1] = 0
        gidx = np.tile(
            ridx.reshape(gtot, 8, 16).transpose(2, 0, 1).reshape(16, gtot * 8),
            (8, 1)).astype(np.int16)

        # agg init = self-loop contribution xs^T for this core's windows
        xslocT = np.zeros((C, npc_out), np.float32)
        dloc = np.zeros(nwin * P, np.float32)
        for j in range(nslots):
            w = win_of[c, j]
            if w < 0:
                continue
            lo = w * AW
            ws = min(AW, N - lo)
            xslocT[:, j * AW:j * AW + ws] = xs[lo:lo + ws].T
            dloc[j * AW:j * AW + ws] = dinv[lo:lo + ws]
        dinvloc = np.ascontiguousarray(dloc.reshape(nwin, P).T)

        in_maps.append({
            "xs": xs_pack,
            "gidx": np.ascontiguousarray(gidx),
            "dstrel": np.ascontiguousarray(drel.T.astype(BF16)),
            "xslocT": xslocT,
            "dinvloc": dinvloc,
            "wmat": W32,
            "bias": b32,
        })
    meta = {
        "G_bw": G_bw,
        "calls": calls,
        "group_meta": group_meta,
        "gstart": gstart,
        "gtot": gtot,
        "nslots": nslots,
        "nwin": nwin,
        "npc_out": npc_out,
        "n_bands": n_bands,
        "band_rows": band_rows,
    }
    return in_maps, meta, win_of


def _build_program(meta, N, C, n_cores):
    from concourse import bacc, bass, mybir, tile

    f32 = mybir.dt.float32
    bf16 = mybir.dt.bfloat16
    i32 = mybir.dt.int32
    i16 = mybir.dt.int16
    gtot = meta["gtot"]
    nslots = meta["nslots"]
    nwin = meta["nwin"]
    npc_out = meta["npc_out"]
    n_bands = meta["n_bands"]
    band_rows = meta["band_rows"]
    calls = meta["calls"]
    group_meta = meta["group_meta"]
    gstart = meta["gstart"]
    G_bw = meta["G_bw"]

    nc = bacc.Bacc("TRN2", target_bir_lowering=False, debug=False,
                   num_devices=n_cores, num_swdge_queues=N_QUEUES,
                   dynamic_dma_scratch_size=32768)
    xs_d = nc.dram_tensor("xs", [N, 2 * C], bf16, kind="ExternalInput")
    gidx_d = nc.dram_tensor("gidx", [P, gtot * 8], i16, kind="ExternalInput")
    dr_d = nc.dram_tensor("dstrel", [P, gtot], bf16, kind="ExternalInput")
    xslocT_d = nc.dram_tensor("xslocT", [C, npc_out], f32,
                              kind="ExternalInput")
    dloc_d = nc.dram_tensor("dinvloc", [P, nwin], f32, kind="ExternalInput")
    w_d = nc.dram_tensor("wmat", [C, C], f32, kind="ExternalInput")
    b_d = nc.dram_tensor("bias", [P, C], f32, kind="ExternalInput")
    out_d = nc.dram_tensor("out", [npc_out, C], f32, kind="ExternalOutput")

    # which band is the last nonempty one per slot (finals emitted there);
    # slots empty in every band get finals in an epilogue
    last_band = {}
    for bb in range(n_bands):
        for j in range(nslots):
            if G_bw[bb, j] > 0:
                last_band[j] = bb
    epilogue_slots = [j for j in range(nslots) if j not in last_band]

    with tile.TileContext(nc) as tc:
        with (
            tc.tile_pool(name="const", bufs=1) as cpool,
            tc.tile_pool(name="aux", bufs=1) as apool,
            tc.tile_pool(name="msg", bufs=3) as mpool,
            tc.tile_pool(name="oh", bufs=3) as ohpool,
            tc.tile_pool(name="flush", bufs=3) as fpool,
            tc.tile_pool(name="agg_ps", bufs=2, space="PSUM") as pspool,
            tc.tile_pool(name="out_ps", bufs=2, space="PSUM") as pspool2,
        ):
            iota_i = cpool.tile([P, AW], i32)
            nc.gpsimd.iota(iota_i[:], pattern=[[1, AW]], base=0,
                           channel_multiplier=0)
            iota_bf = cpool.tile([P, AW], bf16)
            nc.vector.tensor_copy(iota_bf[:], iota_i[:])
            wt = cpool.tile([C, C], f32)
            nc.sync.dma_start(out=wt[:], in_=w_d[:])
            bt = cpool.tile([P, C], f32)
            nc.sync.dma_start(out=bt[:], in_=b_d[:])
            gidx_sb = apool.tile([P, gtot * 8], i16)
            nc.sync.dma_start(out=gidx_sb[:], in_=gidx_d[:])
            dr_sb = apool.tile([P, gtot], bf16)
            nc.sync.dma_start(out=dr_sb[:], in_=dr_d[:])
            dloc_sb = apool.tile([P, nwin], f32)
            nc.sync.dma_start(out=dloc_sb[:], in_=dloc_d[:])
            agg_sb = apool.tile([C, npc_out], f32)
            nc.sync.dma_start(out=agg_sb[:], in_=xslocT_d[:])

            def finals(j):
                for half in range(2):
                    w = j * 2 + half
                    out_ps = pspool2.tile([P, C], f32)
                    nc.tensor.matmul(
                        out_ps[:],
                        lhsT=agg_sb[:, w * WIN:(w + 1) * WIN],
                        rhs=wt[:],
                        start=True,
                        stop=True,
                    )
                    out_sb = fpool.tile([P, C], f32)
                    nc.vector.tensor_scalar(
                        out=out_sb[:], in0=out_ps[:],
                        scalar1=dloc_sb[:, w:w + 1],
                        scalar2=None,
                        op0=mybir.AluOpType.mult)
                    nc.vector.tensor_tensor(
                        out=out_sb[:], in0=out_sb[:], in1=bt[:],
                        op=mybir.AluOpType.add)
                    nc.sync.dma_start(
                        out=out_d[w * WIN:(w + 1) * WIN, :],
                        in_=out_sb[:])

            agg = None
            oh = None
            for ci, (bb, cg0, cng) in enumerate(calls):
                goff = int(gstart[bb, 0])
                msg = mpool.tile([P, KG, 2 * C], bf16)
                lo = bb * band_rows
                hi = min(lo + band_rows, N)
                nc.gpsimd.dma_gather(
                    out_ap=msg[:, :cng, :],
                    in_ap=xs_d[lo:hi, :],
                    idxs_ap=gidx_sb[:, cg0 * 8:(cg0 + cng) * 8],
                    num_idxs=cng * P,
                    num_idxs_reg=cng * P,
                    elem_size=2 * C,
                    single_packet=False,
                    queue_num=ci % N_QUEUES,
                )
                for jj in range(cng):
                    g = cg0 + jj  # global group id
                    if jj % OH_B == 0:
                        nb = min(OH_B, cng - jj)
                        oh = ohpool.tile([P, OH_B, AW], bf16)
                        nc.vector.tensor_tensor(
                            out=oh[:, :nb, :],
                            in0=iota_bf[:, None, :].to_broadcast([P, nb, AW]),
                            in1=dr_sb[:, g:g + nb, None].to_broadcast(
                                [P, nb, AW]),
                            op=mybir.AluOpType.is_equal,
                        )
                    j, k, Gslot = group_meta[bb][g - goff]
                    if k == 0:
                        agg = pspool.tile([C, AW], f32)
                    nc.tensor.matmul(
                        agg[:],
                        lhsT=msg[:, jj, 0:C],
                        rhs=oh[:, jj % OH_B, :],
                        start=(k == 0),
                        stop=(k == Gslot - 1),
                    )
                    if k == Gslot - 1:
                        nc.vector.tensor_tensor(
                            out=agg_sb[:, j * AW:(j + 1) * AW],
                            in0=agg_sb[:, j * AW:(j + 1) * AW],
                            in1=agg[:],
                            op=mybir.AluOpType.add)
                        if last_band.get(j) == bb:
                            finals(j)
            for j in epilogue_slots:
                finals(j)
    nc.compile()
    return nc


_PROGRAM_CACHE = {}


def _run(x, edge_index, W, b, n_cores=N_CORES, band_rows=BAND_ROWS,
         trace=False, sim=False, sim_cores=None):
    in_maps, meta, win_of = _prepare(x, edge_index, W, b, n_cores, band_rows,
                                     pad_neg1=not sim)
    key = (meta["G_bw"].tobytes(), tuple(meta["calls"]), x.shape, sim)
    nc = _PROGRAM_CACHE.get(key)
    if nc is None:
        nc = _build_program(meta, x.shape[0], x.shape[1], n_cores)
        _PROGRAM_CACHE[key] = nc

    N, C = x.shape
    nslots = meta["nslots"]

    if sim:
        from concourse.bass_interp import CoreSim
        outs = {}
        for c in (sim_cores if sim_cores is not None else range(n_cores)):
            s = CoreSim(nc)
            for k, v in in_maps[c].items():
                s.tensor(k)[:] = v
            s.simulate()
            outs[c] = np.array(s.tensor("out"))
        exec_ns = None
    else:
        from concourse.bass_utils import run_bass_kernel_spmd
        res = run_bass_kernel_spmd(nc, in_maps, list(range(n_cores)),
                                   trace=trace)
        outs = {c: res.results[c]["out"] for c in range(n_cores)}
        exec_ns = res.exec_time_ns

    out_full = np.zeros((N, C), np.float32)
    for c, co in outs.items():
        for j in range(nslots):
            w = win_of[c, j]
            if w < 0:
                continue
            lo = w * AW
            ws = min(AW, N - lo)
            out_full[lo:lo + ws] = co[j * AW:j * AW + ws]
    return out_full, exec_ns


def kernel(x, edge_index, W, b):
    out, _ = _run(np.asarray(x), np.asarray(edge_index), np.asarray(W),
                  np.asarray(b))
    return out
